# revision 7
# baseline (speedup 1.0000x reference)
"""Trainium2 Bass kernel for nn_AlgebraicFusion (complex bank mixer).

Algebra (per row l, bank n):
  y_n = P_n x_n            P_n = phase-folded bank_W (host precompute)
  w   = softmax(MLP(mean|x_n|^2))
  c   = sum_n w_n y_n
  out = O' (gamma * c * rsqrt(mean|c|^2 + eps))

v2 layout (per core, 1024 rows):
  - 3 superblocks of (2,3,3) l-tiles; pk streamed once per superblock
    (75.5 MB vs 100.6 MB baseline), oks streamed per (superblock, dh).
  - softmax deferred: unnormalized e_n scale x via diag(e_n) folded into
    the PE transpose (plain matmul with rhs=diag(w) instead of identity);
    denominator folds into the final rsqrt.
  - transposes grouped 4-per-PSUM-bank, copied out 3D in one op (gpsimd).
  - Karatsuba chains c-major per ot with early combine: A,B live -> ct_i
    frees B, then C -> ct_r; PSUM stays within 8 banks (4 acc + 3 str + 1 sm).
  - x'_s = xr'+xi' staged per (ot,n,icc) slice on DVE (not materialized).
  - final Karatsuba vs out_W combos (gamma folded), scale by rsqrt row
    factor, interleave (r,i), DMA out.

Sharding: rows (B*L = 8192) split evenly across 8 cores; weights replicated.
"""
import numpy as np

NB, B, L, D = 4, 4, 2048, 1024
EPS, NORM_EPS = 1e-8, 1e-5
NCORES = 8
ROWS = B * L // NCORES          # 1024 rows per core
LT = ROWS // 128                # 8 l-tiles
SBT = [2, 3, 3]                 # l-tiles per superblock
SBO = [0, 2, 5]                 # tile offset per superblock
NSB = 3
IC = 8                          # i-chunks (contraction 1024 = 8*128)
OT = 8                          # o-tiles
DH = 2                          # d2 halves of 512

XT_BUFS = 8
CT_BUFS = 16
PKT_BUFS = 6
OKS_BUFS = 3
XU_BUFS = 24
RAW_BUFS = 2
STG_BUFS = 3


def build_program():
    import concourse.bacc as bacc
    import concourse.tile as tile
    import concourse.mybir as mybir

    AF = mybir.ActivationFunctionType
    from concourse.alu_op_type import AluOpType

    f16 = mybir.dt.float16
    f32 = mybir.dt.float32
    f8 = mybir.dt.float8e4

    nc = bacc.Bacc("TRN2", target_bir_lowering=False, debug=False,
                   num_devices=NCORES)

    xin = nc.dram_tensor("xin", [NB, ROWS, 2 * D], f32, kind="ExternalInput").ap()
    pk = nc.dram_tensor("pk", [OT, NB, 3, 128, 1024], f16,
                        kind="ExternalInput").ap()
    okk = nc.dram_tensor("okk", [DH, 3, 128, OT * 512], f16,
                         kind="ExternalInput").ap()
    w1t = nc.dram_tensor("w1t", [4, 16], f16, kind="ExternalInput").ap()
    b1c = nc.dram_tensor("b1c", [16, 1], f32, kind="ExternalInput").ap()
    w2t = nc.dram_tensor("w2t", [16, 4], f16, kind="ExternalInput").ap()
    b2c = nc.dram_tensor("b2c", [4, 1], f32, kind="ExternalInput").ap()
    eye16d = nc.dram_tensor("eye16", [128, 128], f16, kind="ExternalInput").ap()
    one1d = nc.dram_tensor("one1", [1, 1], f32, kind="ExternalInput").ap()
    outd = nc.dram_tensor("out", [ROWS, 2 * D], f32, kind="ExternalOutput").ap()

    with tile.TileContext(nc) as tc:
        import contextlib
        with contextlib.ExitStack() as ctx:
            cst = ctx.enter_context(tc.tile_pool(name="cst", bufs=1))
            wk = ctx.enter_context(tc.tile_pool(name="wk", bufs=1))
            psacc = ctx.enter_context(tc.tile_pool(name="psacc", bufs=4,
                                                   space="PSUM"))
            pstr = ctx.enter_context(tc.tile_pool(name="pstr", bufs=3,
                                                  space="PSUM"))
            pssm = ctx.enter_context(tc.tile_pool(name="pssm", bufs=1,
                                                  space="PSUM"))

            # state shared across phases
            st = {"xt": {}, "ct": {}, "xu": {}, "wsc": {}}

            def p12_tp(sb):
                """scaled transposes into fresh xt tiles for sb."""
                BN = SBT[sb] * 128
                xt = {}
                for n in range(NB):
                    for comp in (0, 1):
                        xt[(n, comp)] = wk.tile(
                            [128, IC * BN], f16, tag="xt", bufs=XT_BUFS,
                            name=f"xt{sb}_{n}_{comp}")
                st["xt"] = xt
                for tb in range(SBT[sb]):
                    t = SBO[sb] + tb
                    xu = st["xu"].pop(t)
                    wsc = st["wsc"].pop(t)
                    for n in range(NB):
                        diagw = wk.tile([128, 128], f16, tag="diagw", bufs=2,
                                        name=f"dw{t}_{n}")
                        nc.vector.tensor_scalar_mul(
                            diagw[:], eye16[:], wsc[:, n:n + 1])
                        for comp in (0, 1):
                            for icq in range(IC // 4):
                                psq = pstr.tile([128, 512], f32, tag="str",
                                                name=f"psq{t}_{n}_{comp}_{icq}")
                                for k in range(4):
                                    icc = icq * 4 + k
                                    nc.tensor.matmul(
                                        psq[:, k * 128:(k + 1) * 128],
                                        xu[(n, comp)][:, icc * 128:(icc + 1) * 128],
                                        diagw[:],
                                        start=(k == 0), stop=(k == 3))
                                dst = xt[(n, comp)][:].rearrange(
                                    "p (i l) -> p i l", i=IC)[
                                    :, icq * 4:(icq + 1) * 4,
                                    tb * 128:(tb + 1) * 128]
                                src = psq[:].rearrange("p (q l) -> p q l", q=4)
                                nc.vector.tensor_copy(dst, src)

            def pk_dma(sb, ot_):
                pkt = {}
                for c in range(3):
                    for n in range(NB):
                        pt = wk.tile([128, 1024], f16, tag="pkt",
                                     bufs=PKT_BUFS, name=f"pkt{sb}_{ot_}_{c}_{n}")
                        nc.sync.dma_start(pt[:], pk[ot_, n, c])
                        pkt[(c, n)] = pt
                return pkt

            def p3_ot(sb, ot_, pkt):
                """bank matmul chains for one output tile."""
                BN = SBT[sb] * 128
                xt = st["xt"]
                # A: sum Pr^T (xr'+xi')
                psA = psacc.tile([128, BN], f32, tag="acc", name=f"psA{sb}_{ot_}")
                first = True
                for n in range(NB):
                    for icc in range(IC):
                        stg = wk.tile([128, BN], f16, tag="stg", bufs=STG_BUFS,
                                      name=f"stg{sb}_{ot_}_{n}_{icc}")
                        sl = slice(icc * BN, (icc + 1) * BN)
                        nc.vector.tensor_add(stg[:], xt[(n, 0)][:, sl],
                                             xt[(n, 1)][:, sl])
                        nc.tensor.matmul(
                            psA[:], pkt[(0, n)][:, icc * 128:(icc + 1) * 128],
                            stg[:], start=first,
                            stop=(n == NB - 1 and icc == IC - 1))
                        first = False
                aA = wk.tile([128, BN], f32, tag="aA", bufs=2,
                             name=f"aA{sb}_{ot_}")
                nc.vector.tensor_copy(aA[:], psA[:])
                # B: sum (Pi-Pr)^T xr'
                psB = psacc.tile([128, BN], f32, tag="acc", name=f"psB{sb}_{ot_}")
                first = True
                for n in range(NB):
                    for icc in range(IC):
                        nc.tensor.matmul(
                            psB[:], pkt[(1, n)][:, icc * 128:(icc + 1) * 128],
                            xt[(n, 0)][:, icc * BN:(icc + 1) * BN], start=first,
                            stop=(n == NB - 1 and icc == IC - 1))
                        first = False
                cti = wk.tile([128, BN], f16, tag="ct", bufs=CT_BUFS,
                              name=f"cti{sb}_{ot_}")
                nc.vector.tensor_add(cti[:], aA[:], psB[:])
                st["ct"][(1, ot_)] = cti
                # C: sum (Pr+Pi)^T xi'
                psC = psacc.tile([128, BN], f32, tag="acc", name=f"psC{sb}_{ot_}")
                first = True
                for n in range(NB):
                    for icc in range(IC):
                        nc.tensor.matmul(
                            psC[:], pkt[(2, n)][:, icc * 128:(icc + 1) * 128],
                            xt[(n, 1)][:, icc * BN:(icc + 1) * BN], start=first,
                            stop=(n == NB - 1 and icc == IC - 1))
                        first = False
                ctr = wk.tile([128, BN], f16, tag="ct", bufs=CT_BUFS,
                              name=f"ctr{sb}_{ot_}")
                nc.vector.tensor_sub(ctr[:], aA[:], psC[:])
                st["ct"][(0, ot_)] = ctr

            def p4(sb):
                """row sum-of-squares -> rsqrt factor."""
                BN = SBT[sb] * 128
                off = SBO[sb] * 128
                ssp = pssm.tile([1, BN], f32, tag="sm", name=f"ssp{sb}")
                first = True
                for ot_ in range(OT):
                    for c in (0, 1):
                        c2 = wk.tile([128, BN], f16, tag="c2", bufs=2,
                                     name=f"c2{sb}_{ot_}_{c}")
                        nc.scalar.activation(c2[:], st["ct"][(c, ot_)][:],
                                             AF.Square)
                        nc.tensor.matmul(ssp[:], onesD[:], c2[:], start=first,
                                         stop=(ot_ == OT - 1 and c == 1))
                        first = False
                sms = wk.tile([1, BN], f32, tag="sms", bufs=2, name=f"sms{sb}")
                nc.vector.scalar_tensor_tensor(
                    sms[:], sseps[0:1, off:off + BN], NORM_EPS, ssp[:],
                    AluOpType.mult, AluOpType.add)
                rec = wk.tile([1, BN], f32, tag="rec", bufs=2, name=f"rec{sb}")
                nc.vector.reciprocal(rec[:], sms[:])
                nc.scalar.activation(invrow[0:1, off:off + BN], rec[:], AF.Sqrt)

            def p5(sb, oks):
                """final matmuls + scale + interleave + out DMA (dh-outer)."""
                ct = st["ct"]
                ctSs, invcs = [], []
                for tb in range(SBT[sb]):
                    t = SBO[sb] + tb
                    ctS = wk.tile([128, OT * 128], f16, tag="ctS", bufs=3,
                                  name=f"ctS{t}")
                    for oc in range(OT):
                        nc.vector.tensor_add(
                            ctS[:, oc * 128:(oc + 1) * 128],
                            ct[(0, oc)][:, tb * 128:(tb + 1) * 128],
                            ct[(1, oc)][:, tb * 128:(tb + 1) * 128])
                    ctSs.append(ctS)
                    invp = pssm.tile([128, 1], f32, tag="sm", name=f"invp{t}")
                    nc.tensor.transpose(invp[:],
                                        invrow[0:1, t * 128:(t + 1) * 128],
                                        one1[:])
                    invc = wk.tile([128, 1], f32, tag="invc", bufs=4,
                                   name=f"invc{t}")
                    nc.vector.tensor_copy(invc[:], invp[:])
                    invcs.append(invc)
                for dh in range(DH):
                    if dh == 1:
                        for okc in range(3):
                            ok = wk.tile([128, OT * 512], f16, tag="oks",
                                         bufs=OKS_BUFS,
                                         name=f"oks{sb}_1_{okc}")
                            nc.gpsimd.dma_start(ok[:], okk[1, okc])
                            oks[(1, okc)] = ok
                    for tb in range(SBT[sb]):
                        t = SBO[sb] + tb
                        fA = psacc.tile([128, 512], f32, tag="acc",
                                        name=f"fA{t}_{dh}")
                        for oc in range(OT):
                            nc.tensor.matmul(
                                fA[:], ctSs[tb][:, oc * 128:(oc + 1) * 128],
                                oks[(dh, 0)][:, oc * 512:(oc + 1) * 512],
                                start=(oc == 0), stop=(oc == OT - 1))
                        fa = wk.tile([128, 512], f32, tag="fa", bufs=1,
                                     name=f"fa{t}_{dh}")
                        nc.vector.tensor_copy(fa[:], fA[:])
                        fB = psacc.tile([128, 512], f32, tag="acc",
                                        name=f"fB{t}_{dh}")
                        for oc in range(OT):
                            nc.tensor.matmul(
                                fB[:], ct[(0, oc)][:, tb * 128:(tb + 1) * 128],
                                oks[(dh, 1)][:, oc * 512:(oc + 1) * 512],
                                start=(oc == 0), stop=(oc == OT - 1))
                        fi = wk.tile([128, 512], f32, tag="fi", bufs=1,
                                     name=f"fi{t}_{dh}")
                        nc.vector.tensor_add(fi[:], fa[:], fB[:])
                        fC = psacc.tile([128, 512], f32, tag="acc",
                                        name=f"fC{t}_{dh}")
                        for oc in range(OT):
                            nc.tensor.matmul(
                                fC[:], ct[(1, oc)][:, tb * 128:(tb + 1) * 128],
                                oks[(dh, 2)][:, oc * 512:(oc + 1) * 512],
                                start=(oc == 0), stop=(oc == OT - 1))
                        fr = wk.tile([128, 512], f32, tag="fr", bufs=1,
                                     name=f"fr{t}_{dh}")
                        nc.vector.tensor_sub(fr[:], fa[:], fC[:])
                        oto = wk.tile([128, 1024], f32, tag="oto", bufs=2,
                                      name=f"oto{t}_{dh}")
                        ov = oto[:].rearrange("p (d c) -> p c d", c=2)
                        nc.scalar.activation(ov[:, 0], fr[:], AF.Copy,
                                             scale=invcs[tb][:])
                        nc.scalar.activation(ov[:, 1], fi[:], AF.Copy,
                                             scale=invcs[tb][:])
                        nc.gpsimd.dma_start(
                            outd[t * 128:(t + 1) * 128,
                                 dh * 1024:(dh + 1) * 1024], oto[:])

            # ================= program =================
            p12_pending = [None]  # placeholder so first loads happen below
            # first two raw-tile loads must hit the DMA queue before weights
            raws_first = []
            for tb in range(SBT[0]):
                t = SBO[0] + tb
                tile_raws = []
                for n in range(NB):
                    raw = wk.tile([128, 2 * D], f32, tag="raw", bufs=RAW_BUFS,
                                  name=f"raw{t}_{n}")
                    nc.sync.dma_start(raw[:], xin[n, t * 128:(t + 1) * 128, :])
                    tile_raws.append(raw)
                raws_first.append(tile_raws)

            eye16 = cst.tile([128, 128], f16)
            nc.sync.dma_start(eye16[:], eye16d)
            one1 = cst.tile([1, 1], f32)
            nc.sync.dma_start(one1[:], one1d)
            w1s = cst.tile([4, 16], f16)
            nc.sync.dma_start(w1s[:], w1t)
            b1s = cst.tile([16, 1], f32)
            nc.sync.dma_start(b1s[:], b1c)
            w2s = cst.tile([16, 4], f16)
            nc.sync.dma_start(w2s[:], w2t)
            b2s = cst.tile([4, 1], f32)
            nc.sync.dma_start(b2s[:], b2c)
            onesD = cst.tile([128, 1], f16)
            nc.vector.memset(onesD[:], 1.0 / D)
            ones4 = cst.tile([NB, 1], f16)
            nc.vector.memset(ones4[:], 1.0)
            sseps = cst.tile([1, ROWS], f32)   # s^2 per row
            invrow = cst.tile([1, ROWS], f32)  # rsqrt result per row

            def load_act(t, raws=None):
                """raws (if not preloaded), casts, squares, mag for a tile."""
                if raws is None:
                    raws = []
                    for n in range(NB):
                        raw = wk.tile([128, 2 * D], f32, tag="raw",
                                      bufs=RAW_BUFS, name=f"raw{t}_{n}")
                        nc.sync.dma_start(raw[:],
                                          xin[n, t * 128:(t + 1) * 128, :])
                        raws.append(raw)
                xu = {}
                for n in range(NB):
                    rv = raws[n][:].rearrange("p (d c) -> p c d", c=2)
                    for comp in (0, 1):
                        xc = wk.tile([128, D], f16, tag="xu", bufs=XU_BUFS,
                                     name=f"xu{t}_{n}_{comp}")
                        nc.scalar.copy(xc[:], rv[:, comp])
                        xu[(n, comp)] = xc
                st["xu"][t] = xu
                magr = wk.tile([128, NB], f32, tag="magr", bufs=2,
                               name=f"magr{t}")
                magi = wk.tile([128, NB], f32, tag="magi", bufs=2,
                               name=f"magi{t}")
                for n in range(NB):
                    junk = wk.tile([128, D], f8, tag="junk", bufs=1,
                                   name=f"junk{t}_{n}")
                    nc.scalar.activation(junk[:], xu[(n, 0)][:], AF.Square,
                                         accum_out=magr[:, n:n + 1])
                    junk2 = wk.tile([128, D], f8, tag="junk", bufs=1,
                                    name=f"junk2{t}_{n}")
                    nc.scalar.activation(junk2[:], xu[(n, 1)][:], AF.Square,
                                         accum_out=magi[:, n:n + 1])
                mag = wk.tile([128, NB], f16, tag="mag", bufs=3, name=f"mag{t}")
                nc.vector.tensor_add(mag[:], magr[:], magi[:])
                st[f"mag{t}"] = mag

            def router(t):
                """per-tile router MLP -> unnormalized softmax scales wsc."""
                mag = st.pop(f"mag{t}")
                mtp = pssm.tile([NB, 128], f16, tag="sm", name=f"mtp{t}")
                nc.tensor.transpose(mtp[:], mag[:], eye16[:])
                magT = wk.tile([NB, 128], f16, tag="magT", bufs=2,
                               name=f"magT{t}")
                nc.vector.tensor_copy(magT[:], mtp[:])
                h1p = pssm.tile([16, 128], f32, tag="sm", name=f"h1p{t}")
                nc.tensor.matmul(h1p[:], w1s[:], magT[:], start=True, stop=True)
                h1 = wk.tile([16, 128], f16, tag="h1", bufs=2, name=f"h1{t}")
                nc.scalar.activation(h1[:], h1p[:], AF.Gelu, bias=b1s[:])
                lgp = pssm.tile([NB, 128], f32, tag="sm", name=f"lgp{t}")
                nc.tensor.matmul(lgp[:], w2s[:], h1[:], start=True, stop=True)
                e4 = wk.tile([NB, 128], f16, tag="e4", bufs=2, name=f"e4{t}")
                nc.scalar.activation(e4[:], lgp[:], AF.Exp, bias=b2s[:])
                ssump = pssm.tile([1, 128], f32, tag="sm", name=f"ssump{t}")
                nc.tensor.matmul(ssump[:], ones4[:], e4[:], start=True,
                                 stop=True)
                s_sb = wk.tile([1, 128], f32, tag="s_sb", bufs=2,
                               name=f"ssb{t}")
                nc.vector.tensor_copy(s_sb[:], ssump[:])
                nc.vector.tensor_mul(sseps[0:1, t * 128:(t + 1) * 128],
                                     s_sb[:], s_sb[:])
                eTp = pssm.tile([128, NB], f16, tag="sm", name=f"eTp{t}")
                nc.tensor.transpose(eTp[:], e4[:], eye16[0:NB, 0:NB])
                wsc = wk.tile([128, NB], f32, tag="wsc", bufs=4, name=f"wsc{t}")
                nc.vector.tensor_copy(wsc[:], eTp[:])
                st["wsc"][t] = wsc

            pkt_next = [pk_dma(0, 0), pk_dma(0, 1)]
            for tb in range(SBT[0]):
                load_act(SBO[0] + tb, raws_first[tb])
            for tb in range(SBT[0]):
                router(SBO[0] + tb)
            p12_tp(0)
            for sb in range(NSB):
                nxt = sb + 1
                for ot_ in range(OT):
                    if nxt < NSB and ot_ < SBT[nxt]:
                        load_act(SBO[nxt] + ot_)
                    if nxt < NSB and 2 <= ot_ < 2 + SBT[nxt]:
                        router(SBO[nxt] + ot_ - 2)
                    pkt = pkt_next.pop(0)
                    if ot_ + 2 < OT:
                        pkt_next.append(pk_dma(sb, ot_ + 2))
                    elif nxt < NSB:
                        pkt_next.append(pk_dma(nxt, ot_ + 2 - OT))
                    p3_ot(sb, ot_, pkt)
                oks = {}
                for okc in range(3):
                    ok = wk.tile([128, OT * 512], f16, tag="oks",
                                 bufs=OKS_BUFS, name=f"oks{sb}_0_{okc}")
                    nc.gpsimd.dma_start(ok[:], okk[0, okc])
                    oks[(0, okc)] = ok
                if nxt < NSB:
                    p12_tp(nxt)
                p4(sb)
                p5(sb, oks)

    nc.compile()
    return nc


def host_prep(inputs):
    """Build per-core in_maps from full inputs (numpy f32)."""
    f16 = np.float16
    phase = np.asarray(inputs["phase"], np.float32)
    bank_W = np.asarray(inputs["bank_W"], np.float32)
    W1 = np.asarray(inputs["W1"], np.float32)
    b1 = np.asarray(inputs["b1"], np.float32)
    W2 = np.asarray(inputs["W2"], np.float32)
    b2 = np.asarray(inputs["b2"], np.float32)
    gamma = np.asarray(inputs["gamma"], np.float32)
    out_W = np.asarray(inputs["out_W"], np.float32)
    bank_out = np.asarray(inputs["bank_out"], np.float32)

    pr, pi = phase[..., 0], phase[..., 1]
    pm = np.sqrt(pr * pr + pi * pi) + EPS
    ur, ui = (pr / pm)[:, :, None], (pi / pm)[:, :, None]
    Wr, Wi = bank_W[..., 0], bank_W[..., 1]
    Pr = Wr * ur - Wi * ui
    Pi_ = Wr * ui + Wi * ur
    KT = np.stack([Pr, Pi_ - Pr, Pr + Pi_], 1).transpose(0, 1, 3, 2)  # [n,c,i,o]
    # pk[ot, n, c, iw, ic*128+ow] = KT[n, c, ic*128+iw, ot*128+ow]
    pkarr = np.ascontiguousarray(
        KT.reshape(NB, 3, IC, 128, OT, 128).transpose(4, 0, 1, 3, 2, 5)
        .reshape(OT, NB, 3, 128, 1024).astype(f16))

    Og = out_W * gamma[None, :, None]          # scale c-dim (col index)
    Or, Oi = Og[..., 0], Og[..., 1]
    OKT = np.stack([Or, Oi - Or, Or + Oi], 0).transpose(0, 2, 1)  # [c, i, d2]
    # okk[dh, c, ow, oc*512+d2w] = OKT[c, oc*128+ow, dh*512+d2w]
    okarr = np.ascontiguousarray(
        OKT.reshape(3, OT, 128, DH, 512).transpose(3, 0, 2, 1, 4)
        .reshape(DH, 3, 128, OT * 512).astype(f16))

    w1tb = np.ascontiguousarray((W1 / D).T.astype(f16))             # [4, 16]
    b1cb = np.ascontiguousarray(b1[:, None].astype(np.float32))     # [16, 1]
    w2tb = np.ascontiguousarray(W2.T.astype(f16))                   # [16, 4]
    b2cb = np.ascontiguousarray(b2[:, None].astype(np.float32))     # [4, 1]
    eye16 = np.eye(128, dtype=f16)
    one1 = np.ones((1, 1), dtype=np.float32)

    xall = bank_out.reshape(NB, B * L, 2 * D)
    shared = dict(pk=pkarr, okk=okarr, w1t=w1tb, b1c=b1cb, w2t=w2tb, b2c=b2cb,
                  eye16=eye16, one1=one1)
    in_maps = []
    for k in range(NCORES):
        xin = np.ascontiguousarray(xall[:, k * ROWS:(k + 1) * ROWS, :])
        in_maps.append(dict(shared, xin=xin))
    return in_maps


_nc_cache = {}


def kernel(**inputs):
    from concourse.bass_utils import run_bass_kernel_spmd

    if "nc" not in _nc_cache:
        _nc_cache["nc"] = build_program()
    nc = _nc_cache["nc"]
    in_maps = host_prep(inputs)
    res = run_bass_kernel_spmd(nc, in_maps, core_ids=list(range(NCORES)))
    out = np.concatenate([r["out"] for r in res.results], axis=0)
    return np.ascontiguousarray(out.reshape(B, L, D, 2))


# revision 9
# speedup vs baseline: 1.0611x; 1.0611x over previous
"""Trainium2 Bass kernel for nn_AlgebraicFusion (complex bank mixer).

Algebra (per row l, bank n):
  y_n = P_n x_n            P_n = phase-folded bank_W (host precompute)
  w   = softmax(MLP(mean|x_n|^2))
  c   = sum_n w_n y_n
  out = O' (gamma * c * rsqrt(mean|c|^2 + eps))

v2 layout (per core, 1024 rows):
  - 3 superblocks of (2,3,3) l-tiles; pk streamed once per superblock
    (75.5 MB vs 100.6 MB baseline), oks streamed per (superblock, dh).
  - softmax deferred: unnormalized e_n scale x via diag(e_n) folded into
    the PE transpose (plain matmul with rhs=diag(w) instead of identity);
    denominator folds into the final rsqrt.
  - transposes grouped 4-per-PSUM-bank, copied out 3D in one op (gpsimd).
  - Karatsuba chains c-major per ot with early combine: A,B live -> ct_i
    frees B, then C -> ct_r; PSUM stays within 8 banks (4 acc + 3 str + 1 sm).
  - x'_s = xr'+xi' staged per (ot,n,icc) slice on DVE (not materialized).
  - final Karatsuba vs out_W combos (gamma folded), scale by rsqrt row
    factor, interleave (r,i), DMA out.

Sharding: rows (B*L = 8192) split evenly across 8 cores; weights replicated.
"""
import numpy as np

NB, B, L, D = 4, 4, 2048, 1024
EPS, NORM_EPS = 1e-8, 1e-5
NCORES = 8
ROWS = B * L // NCORES          # 1024 rows per core
LT = ROWS // 128                # 8 l-tiles
SBT = [2, 3, 3]                 # l-tiles per superblock
SBO = [0, 2, 5]                 # tile offset per superblock
NSB = 3
IC = 8                          # i-chunks (contraction 1024 = 8*128)
OT = 8                          # o-tiles
DH = 2                          # d2 halves of 512

XT_BUFS = 8
CT_BUFS = 16
PKT_BUFS = 4
OKS_BUFS = 3
XU_BUFS = 24
RAW_BUFS = 2
STG_BUFS = 3


def build_program():
    import concourse.bacc as bacc
    import concourse.tile as tile
    import concourse.mybir as mybir

    AF = mybir.ActivationFunctionType
    from concourse.alu_op_type import AluOpType

    f16 = mybir.dt.float16
    f32 = mybir.dt.float32
    f8 = mybir.dt.float8e4

    nc = bacc.Bacc("TRN2", target_bir_lowering=False, debug=False,
                   num_devices=NCORES)

    xin = nc.dram_tensor("xin", [NB, 2, ROWS, D], f16, kind="ExternalInput").ap()
    pk = nc.dram_tensor("pk", [OT, NB, 3, 128, 1024], f16,
                        kind="ExternalInput").ap()
    okk = nc.dram_tensor("okk", [DH, 3, 128, OT * 512], f16,
                         kind="ExternalInput").ap()
    w1t = nc.dram_tensor("w1t", [4, 16], f16, kind="ExternalInput").ap()
    b1c = nc.dram_tensor("b1c", [16, 1], f32, kind="ExternalInput").ap()
    w2t = nc.dram_tensor("w2t", [16, 4], f16, kind="ExternalInput").ap()
    b2c = nc.dram_tensor("b2c", [4, 1], f32, kind="ExternalInput").ap()
    eye16d = nc.dram_tensor("eye16", [128, 128], f16, kind="ExternalInput").ap()
    one1d = nc.dram_tensor("one1", [1, 1], f32, kind="ExternalInput").ap()
    outd = nc.dram_tensor("out", [ROWS, 2 * D], f32, kind="ExternalOutput").ap()

    with tile.TileContext(nc) as tc:
        import contextlib
        with contextlib.ExitStack() as ctx:
            cst = ctx.enter_context(tc.tile_pool(name="cst", bufs=1))
            wk = ctx.enter_context(tc.tile_pool(name="wk", bufs=1))
            psacc = ctx.enter_context(tc.tile_pool(name="psacc", bufs=4,
                                                   space="PSUM"))
            pstr = ctx.enter_context(tc.tile_pool(name="pstr", bufs=3,
                                                  space="PSUM"))
            pssm = ctx.enter_context(tc.tile_pool(name="pssm", bufs=1,
                                                  space="PSUM"))

            # state shared across phases
            st = {"xt": {}, "ct": {}, "xu": {}, "wsc": {}}

            def p12_tp(sb):
                """scaled transposes into fresh xt tiles for sb."""
                BN = SBT[sb] * 128
                xt = {}
                for n in range(NB):
                    for comp in (0, 1):
                        xt[(n, comp)] = wk.tile(
                            [128, IC * BN], f16, tag="xt", bufs=XT_BUFS,
                            name=f"xt{sb}_{n}_{comp}")
                st["xt"] = xt
                xus = {t: st["xu"].pop(t)
                       for t in range(SBO[sb], SBO[sb] + SBT[sb])}
                wscs = {t: st["wsc"].pop(t)
                        for t in range(SBO[sb], SBO[sb] + SBT[sb])}
                xts = {}
                for n in range(NB):
                    for tb in range(SBT[sb]):
                        t = SBO[sb] + tb
                        diagw = wk.tile([128, 128], f16, tag="diagw", bufs=2,
                                        name=f"dw{t}_{n}")
                        nc.vector.tensor_scalar_mul(
                            diagw[:], eye16[:], wscs[t][:, n:n + 1])
                        for comp in (0, 1):
                            for icq in range(IC // 4):
                                psq = pstr.tile([128, 512], f32, tag="str",
                                                name=f"psq{t}_{n}_{comp}_{icq}")
                                for k in range(4):
                                    icc = icq * 4 + k
                                    nc.tensor.matmul(
                                        psq[:, k * 128:(k + 1) * 128],
                                        xus[t][(n, comp)][:, icc * 128:(icc + 1) * 128],
                                        diagw[:],
                                        start=(k == 0), stop=(k == 3))
                                dst = xt[(n, comp)][:].rearrange(
                                    "p (i l) -> p i l", i=IC)[
                                    :, icq * 4:(icq + 1) * 4,
                                    tb * 128:(tb + 1) * 128]
                                src = psq[:].rearrange("p (q l) -> p q l", q=4)
                                nc.scalar.copy(dst, src)
                    xs = wk.tile([128, IC * BN], f16, tag="xts", bufs=4,
                                 name=f"xts{sb}_{n}")
                    nc.gpsimd.tensor_add(xs[:], xt[(n, 0)][:], xt[(n, 1)][:])
                    xts[n] = xs
                st["xts"] = xts

            def pk_dma(sb, ot_):
                pkt = {}
                for c in range(3):
                    for n in range(NB):
                        pt = wk.tile([128, 1024], f16, tag="pkt",
                                     bufs=PKT_BUFS, name=f"pkt{sb}_{ot_}_{c}_{n}")
                        nc.sync.dma_start(pt[:], pk[ot_, n, c])
                        pkt[(c, n)] = pt
                return pkt

            def p3_ot(sb, ot_, pkt):
                """bank matmul chains for one output tile."""
                BN = SBT[sb] * 128
                xt = st["xt"]
                xts = st["xts"]
                # A: sum Pr^T (xr'+xi')
                psA = psacc.tile([128, BN], f32, tag="acc", name=f"psA{sb}_{ot_}")
                first = True
                for n in range(NB):
                    for icc in range(IC):
                        nc.tensor.matmul(
                            psA[:], pkt[(0, n)][:, icc * 128:(icc + 1) * 128],
                            xts[n][:, icc * BN:(icc + 1) * BN], start=first,
                            stop=(n == NB - 1 and icc == IC - 1))
                        first = False
                aA = wk.tile([128, BN], f32, tag="aA", bufs=2,
                             name=f"aA{sb}_{ot_}")
                nc.vector.tensor_copy(aA[:], psA[:])
                # B: sum (Pi-Pr)^T xr'
                psB = psacc.tile([128, BN], f32, tag="acc", name=f"psB{sb}_{ot_}")
                first = True
                for n in range(NB):
                    for icc in range(IC):
                        nc.tensor.matmul(
                            psB[:], pkt[(1, n)][:, icc * 128:(icc + 1) * 128],
                            xt[(n, 0)][:, icc * BN:(icc + 1) * BN], start=first,
                            stop=(n == NB - 1 and icc == IC - 1))
                        first = False
                cti = wk.tile([128, BN], f16, tag="ct", bufs=CT_BUFS,
                              name=f"cti{sb}_{ot_}")
                nc.vector.tensor_add(cti[:], aA[:], psB[:])
                st["ct"][(1, ot_)] = cti
                # C: sum (Pr+Pi)^T xi'
                psC = psacc.tile([128, BN], f32, tag="acc", name=f"psC{sb}_{ot_}")
                first = True
                for n in range(NB):
                    for icc in range(IC):
                        nc.tensor.matmul(
                            psC[:], pkt[(2, n)][:, icc * 128:(icc + 1) * 128],
                            xt[(n, 1)][:, icc * BN:(icc + 1) * BN], start=first,
                            stop=(n == NB - 1 and icc == IC - 1))
                        first = False
                ctr = wk.tile([128, BN], f16, tag="ct", bufs=CT_BUFS,
                              name=f"ctr{sb}_{ot_}")
                nc.vector.tensor_sub(ctr[:], aA[:], psC[:])
                st["ct"][(0, ot_)] = ctr

            def p4(sb):
                """row sum-of-squares -> rsqrt factor."""
                BN = SBT[sb] * 128
                off = SBO[sb] * 128
                ssp = pssm.tile([1, BN], f32, tag="sm", name=f"ssp{sb}")
                first = True
                for ot_ in range(OT):
                    for c in (0, 1):
                        c2 = wk.tile([128, BN], f16, tag="c2", bufs=2,
                                     name=f"c2{sb}_{ot_}_{c}")
                        nc.vector.tensor_mul(c2[:], st["ct"][(c, ot_)][:],
                                             st["ct"][(c, ot_)][:])
                        nc.tensor.matmul(ssp[:], onesD[:], c2[:], start=first,
                                         stop=(ot_ == OT - 1 and c == 1))
                        first = False
                sms = wk.tile([1, BN], f32, tag="sms", bufs=2, name=f"sms{sb}")
                nc.vector.scalar_tensor_tensor(
                    sms[:], sseps[0:1, off:off + BN], NORM_EPS, ssp[:],
                    AluOpType.mult, AluOpType.add)
                rec = wk.tile([1, BN], f32, tag="rec", bufs=2, name=f"rec{sb}")
                nc.vector.reciprocal(rec[:], sms[:])
                nc.scalar.activation(invrow[0:1, off:off + BN], rec[:], AF.Sqrt)

            def p5(sb, oks):
                """final matmuls + scale + interleave + out DMA (dh-outer)."""
                ct = st["ct"]
                ctSs, invcs = [], []
                for tb in range(SBT[sb]):
                    t = SBO[sb] + tb
                    ctS = wk.tile([128, OT * 128], f16, tag="ctS", bufs=3,
                                  name=f"ctS{t}")
                    for oc in range(OT):
                        nc.vector.tensor_add(
                            ctS[:, oc * 128:(oc + 1) * 128],
                            ct[(0, oc)][:, tb * 128:(tb + 1) * 128],
                            ct[(1, oc)][:, tb * 128:(tb + 1) * 128])
                    ctSs.append(ctS)
                    invp = pssm.tile([128, 1], f32, tag="sm", name=f"invp{t}")
                    nc.tensor.transpose(invp[:],
                                        invrow[0:1, t * 128:(t + 1) * 128],
                                        one1[:])
                    invc = wk.tile([128, 1], f32, tag="invc", bufs=4,
                                   name=f"invc{t}")
                    nc.vector.tensor_copy(invc[:], invp[:])
                    invcs.append(invc)
                for dh in range(DH):
                    if dh == 1:
                        for okc in range(3):
                            ok = wk.tile([128, OT * 512], f16, tag="oks",
                                         bufs=OKS_BUFS,
                                         name=f"oks{sb}_1_{okc}")
                            nc.gpsimd.dma_start(ok[:], okk[1, okc])
                            oks[(1, okc)] = ok
                    for tb in range(SBT[sb]):
                        t = SBO[sb] + tb
                        fA = psacc.tile([128, 512], f32, tag="acc",
                                        name=f"fA{t}_{dh}")
                        for oc in range(OT):
                            nc.tensor.matmul(
                                fA[:], ctSs[tb][:, oc * 128:(oc + 1) * 128],
                                oks[(dh, 0)][:, oc * 512:(oc + 1) * 512],
                                start=(oc == 0), stop=(oc == OT - 1))
                        fa = wk.tile([128, 512], f32, tag="fa", bufs=1,
                                     name=f"fa{t}_{dh}")
                        nc.vector.tensor_copy(fa[:], fA[:])
                        fB = psacc.tile([128, 512], f32, tag="acc",
                                        name=f"fB{t}_{dh}")
                        for oc in range(OT):
                            nc.tensor.matmul(
                                fB[:], ct[(0, oc)][:, tb * 128:(tb + 1) * 128],
                                oks[(dh, 1)][:, oc * 512:(oc + 1) * 512],
                                start=(oc == 0), stop=(oc == OT - 1))
                        fi = wk.tile([128, 512], f32, tag="fi", bufs=1,
                                     name=f"fi{t}_{dh}")
                        nc.vector.tensor_add(fi[:], fa[:], fB[:])
                        fC = psacc.tile([128, 512], f32, tag="acc",
                                        name=f"fC{t}_{dh}")
                        for oc in range(OT):
                            nc.tensor.matmul(
                                fC[:], ct[(1, oc)][:, tb * 128:(tb + 1) * 128],
                                oks[(dh, 2)][:, oc * 512:(oc + 1) * 512],
                                start=(oc == 0), stop=(oc == OT - 1))
                        fr = wk.tile([128, 512], f32, tag="fr", bufs=1,
                                     name=f"fr{t}_{dh}")
                        nc.vector.tensor_sub(fr[:], fa[:], fC[:])
                        oto = wk.tile([128, 1024], f32, tag="oto", bufs=2,
                                      name=f"oto{t}_{dh}")
                        ov = oto[:].rearrange("p (d c) -> p c d", c=2)
                        nc.scalar.activation(ov[:, 0], fr[:], AF.Copy,
                                             scale=invcs[tb][:])
                        nc.scalar.activation(ov[:, 1], fi[:], AF.Copy,
                                             scale=invcs[tb][:])
                        nc.gpsimd.dma_start(
                            outd[t * 128:(t + 1) * 128,
                                 dh * 1024:(dh + 1) * 1024], oto[:])

            # ================= program =================
            p12_pending = [None]  # placeholder so first loads happen below
            eye16 = cst.tile([128, 128], f16)
            nc.sync.dma_start(eye16[:], eye16d)
            one1 = cst.tile([1, 1], f32)
            nc.sync.dma_start(one1[:], one1d)
            w1s = cst.tile([4, 16], f16)
            nc.sync.dma_start(w1s[:], w1t)
            b1s = cst.tile([16, 1], f32)
            nc.sync.dma_start(b1s[:], b1c)
            w2s = cst.tile([16, 4], f16)
            nc.sync.dma_start(w2s[:], w2t)
            b2s = cst.tile([4, 1], f32)
            nc.sync.dma_start(b2s[:], b2c)
            onesD = cst.tile([128, 1], f16)
            nc.vector.memset(onesD[:], 1.0 / D)
            ones4 = cst.tile([NB, 1], f16)
            nc.vector.memset(ones4[:], 1.0)
            sseps = cst.tile([1, ROWS], f32)   # s^2 per row
            invrow = cst.tile([1, ROWS], f32)  # rsqrt result per row

            def load_act(t):
                """xu DMAs (host pre-cast f16, deinterleaved), squares, mag."""
                xu = {}
                for n in range(NB):
                    for comp in (0, 1):
                        xc = wk.tile([128, D], f16, tag="xu", bufs=XU_BUFS,
                                     name=f"xu{t}_{n}_{comp}")
                        nc.sync.dma_start(
                            xc[:], xin[n, comp, t * 128:(t + 1) * 128, :])
                        xu[(n, comp)] = xc
                st["xu"][t] = xu
                magr = wk.tile([128, NB], f32, tag="magr", bufs=2,
                               name=f"magr{t}")
                magi = wk.tile([128, NB], f32, tag="magi", bufs=2,
                               name=f"magi{t}")
                for n in range(NB):
                    junk = wk.tile([128, D], f8, tag="junk", bufs=1,
                                   name=f"junk{t}_{n}")
                    nc.scalar.activation(junk[:], xu[(n, 0)][:], AF.Square,
                                         accum_out=magr[:, n:n + 1])
                    junk2 = wk.tile([128, D], f8, tag="junk", bufs=1,
                                    name=f"junk2{t}_{n}")
                    nc.scalar.activation(junk2[:], xu[(n, 1)][:], AF.Square,
                                         accum_out=magi[:, n:n + 1])
                mag = wk.tile([128, NB], f16, tag="mag", bufs=3, name=f"mag{t}")
                nc.vector.tensor_add(mag[:], magr[:], magi[:])
                st[f"mag{t}"] = mag

            def router(t):
                """per-tile router MLP -> unnormalized softmax scales wsc."""
                mag = st.pop(f"mag{t}")
                mtp = pssm.tile([NB, 128], f16, tag="sm", name=f"mtp{t}")
                nc.tensor.transpose(mtp[:], mag[:], eye16[:])
                magT = wk.tile([NB, 128], f16, tag="magT", bufs=2,
                               name=f"magT{t}")
                nc.vector.tensor_copy(magT[:], mtp[:])
                h1p = pssm.tile([16, 128], f32, tag="sm", name=f"h1p{t}")
                nc.tensor.matmul(h1p[:], w1s[:], magT[:], start=True, stop=True)
                h1 = wk.tile([16, 128], f16, tag="h1", bufs=2, name=f"h1{t}")
                nc.scalar.activation(h1[:], h1p[:], AF.Gelu, bias=b1s[:])
                lgp = pssm.tile([NB, 128], f32, tag="sm", name=f"lgp{t}")
                nc.tensor.matmul(lgp[:], w2s[:], h1[:], start=True, stop=True)
                e4 = wk.tile([NB, 128], f16, tag="e4", bufs=2, name=f"e4{t}")
                nc.scalar.activation(e4[:], lgp[:], AF.Exp, bias=b2s[:])
                ssump = pssm.tile([1, 128], f32, tag="sm", name=f"ssump{t}")
                nc.tensor.matmul(ssump[:], ones4[:], e4[:], start=True,
                                 stop=True)
                s_sb = wk.tile([1, 128], f32, tag="s_sb", bufs=2,
                               name=f"ssb{t}")
                nc.vector.tensor_copy(s_sb[:], ssump[:])
                nc.vector.tensor_mul(sseps[0:1, t * 128:(t + 1) * 128],
                                     s_sb[:], s_sb[:])
                eTp = pssm.tile([128, NB], f16, tag="sm", name=f"eTp{t}")
                nc.tensor.transpose(eTp[:], e4[:], eye16[0:NB, 0:NB])
                wsc = wk.tile([128, NB], f32, tag="wsc", bufs=4, name=f"wsc{t}")
                nc.vector.tensor_copy(wsc[:], eTp[:])
                st["wsc"][t] = wsc

            for tb in range(SBT[0]):
                load_act(SBO[0] + tb)
            pkt_next = [pk_dma(0, 0), pk_dma(0, 1)]
            for tb in range(SBT[0]):
                router(SBO[0] + tb)
            p12_tp(0)
            for sb in range(NSB):
                nxt = sb + 1
                for ot_ in range(OT):
                    if nxt < NSB and ot_ < SBT[nxt]:
                        load_act(SBO[nxt] + ot_)
                    if nxt < NSB and 2 <= ot_ < 2 + SBT[nxt]:
                        router(SBO[nxt] + ot_ - 2)
                    pkt = pkt_next.pop(0)
                    if ot_ + 2 < OT:
                        pkt_next.append(pk_dma(sb, ot_ + 2))
                    elif nxt < NSB:
                        pkt_next.append(pk_dma(nxt, ot_ + 2 - OT))
                    p3_ot(sb, ot_, pkt)
                oks = {}
                for okc in range(3):
                    ok = wk.tile([128, OT * 512], f16, tag="oks",
                                 bufs=OKS_BUFS, name=f"oks{sb}_0_{okc}")
                    nc.gpsimd.dma_start(ok[:], okk[0, okc])
                    oks[(0, okc)] = ok
                if nxt < NSB:
                    p12_tp(nxt)
                p4(sb)
                p5(sb, oks)

    nc.compile()
    return nc


def host_prep(inputs):
    """Build per-core in_maps from full inputs (numpy f32)."""
    f16 = np.float16
    phase = np.asarray(inputs["phase"], np.float32)
    bank_W = np.asarray(inputs["bank_W"], np.float32)
    W1 = np.asarray(inputs["W1"], np.float32)
    b1 = np.asarray(inputs["b1"], np.float32)
    W2 = np.asarray(inputs["W2"], np.float32)
    b2 = np.asarray(inputs["b2"], np.float32)
    gamma = np.asarray(inputs["gamma"], np.float32)
    out_W = np.asarray(inputs["out_W"], np.float32)
    bank_out = np.asarray(inputs["bank_out"], np.float32)

    pr, pi = phase[..., 0], phase[..., 1]
    pm = np.sqrt(pr * pr + pi * pi) + EPS
    ur, ui = (pr / pm)[:, :, None], (pi / pm)[:, :, None]
    Wr, Wi = bank_W[..., 0], bank_W[..., 1]
    Pr = Wr * ur - Wi * ui
    Pi_ = Wr * ui + Wi * ur
    KT = np.stack([Pr, Pi_ - Pr, Pr + Pi_], 1).transpose(0, 1, 3, 2)  # [n,c,i,o]
    # pk[ot, n, c, iw, ic*128+ow] = KT[n, c, ic*128+iw, ot*128+ow]
    pkarr = np.ascontiguousarray(
        KT.reshape(NB, 3, IC, 128, OT, 128).transpose(4, 0, 1, 3, 2, 5)
        .reshape(OT, NB, 3, 128, 1024).astype(f16))

    Og = out_W * gamma[None, :, None]          # scale c-dim (col index)
    Or, Oi = Og[..., 0], Og[..., 1]
    OKT = np.stack([Or, Oi - Or, Or + Oi], 0).transpose(0, 2, 1)  # [c, i, d2]
    # okk[dh, c, ow, oc*512+d2w] = OKT[c, oc*128+ow, dh*512+d2w]
    okarr = np.ascontiguousarray(
        OKT.reshape(3, OT, 128, DH, 512).transpose(3, 0, 2, 1, 4)
        .reshape(DH, 3, 128, OT * 512).astype(f16))

    w1tb = np.ascontiguousarray((W1 / D).T.astype(f16))             # [4, 16]
    b1cb = np.ascontiguousarray(b1[:, None].astype(np.float32))     # [16, 1]
    w2tb = np.ascontiguousarray(W2.T.astype(f16))                   # [16, 4]
    b2cb = np.ascontiguousarray(b2[:, None].astype(np.float32))     # [4, 1]
    eye16 = np.eye(128, dtype=f16)
    one1 = np.ones((1, 1), dtype=np.float32)

    xall = bank_out.reshape(NB, B * L, D, 2).transpose(0, 3, 1, 2)
    xall = np.ascontiguousarray(xall.astype(f16))       # [NB, 2, B*L, D]
    shared = dict(pk=pkarr, okk=okarr, w1t=w1tb, b1c=b1cb, w2t=w2tb, b2c=b2cb,
                  eye16=eye16, one1=one1)
    in_maps = []
    for k in range(NCORES):
        xin = np.ascontiguousarray(xall[:, :, k * ROWS:(k + 1) * ROWS, :])
        in_maps.append(dict(shared, xin=xin))
    return in_maps


_nc_cache = {}


def kernel(**inputs):
    from concourse.bass_utils import run_bass_kernel_spmd

    if "nc" not in _nc_cache:
        _nc_cache["nc"] = build_program()
    nc = _nc_cache["nc"]
    in_maps = host_prep(inputs)
    res = run_bass_kernel_spmd(nc, in_maps, core_ids=list(range(NCORES)))
    out = np.concatenate([r["out"] for r in res.results], axis=0)
    return np.ascontiguousarray(out.reshape(B, L, D, 2))
